# revision 2
# baseline (speedup 1.0000x reference)
"""FacenetLoss Trainium2 kernel.

Strategy
--------
N=384, D=128.  The reference builds an [N,N,N] triplet tensor; we never
materialize it.  For anchor i, positive j, negative k:

    tl[i,j,k] = relu((d_ij + MARGIN) - d_ik)

with d the squared-euclidean pairwise distance.  Since
d_ij - d_ik = (sq_j - 2 G_ij) - (sq_k - 2 G_ik)  (sq_i cancels, G = E @ E.T),
we compute on device:

    U[i,k] = sq_k - 2*G[i,k]          (bias source: bias(i,j) = U[i,j] + M)
    V[i,k] = -U[i,k] - poison[i,k]    (row source;  poison = BIG where
                                       classes equal -> invalid negatives k
                                       give tl == 0 exactly)

Only same-class (i,j) pairs ever contribute to the loss (rows with
same_group == 0 reduce to exactly 0 in the reference), so we enumerate the
~N*N/C valid pairs on host (integer bookkeeping only; all math derived from
embeddings stays on device), pack them 128 per partition-tile, and for each
pair gather V[i,:] (indirect DMA row gather) and U[i,j] (indirect DMA element
gather).  Per pair-row over k (free axis):

    t    = relu(V_row + bias)                                  [ACT]
    pps  = max_k( t * (t <= M) )                               [DVE]
    sh   = min_k( t - RHO*(t > M) )                            [DVE]
    ppl  = w * ( pps + (pps==0) * (sh+RHO) * (sh<0) )          [DVE, small]

and the scalar partial sum per core is reduced on host across the 8 cores:
loss = sum(ppl) / max(den,1),  den = number of valid pairs (host integer).

Poisoned k (same class, incl. k==i and k==j) give t == 0, which drops out of
pps (max of nonnegatives) and makes sh <= 0 only when the true min is also
<= 0 path-equivalent (t==0 -> term 0; gate (sh<0) yields identical pph).
"""

import functools
import math

import numpy as np

N = 384
D = 128
NB = N // 128  # 3 row blocks
P = 128
NCORES = 8
MARGIN = 0.2
RHO = 10.0
BIG = 1.0e5


@functools.lru_cache(maxsize=4)
def _build(T: int):
    """Compile the SPMD kernel for T pair-tiles per core. Returns nc."""
    from contextlib import ExitStack

    import concourse.bacc as bacc
    import concourse.bass as bass
    import concourse.mybir as mybir
    import concourse.tile as tile
    from concourse.masks import make_identity

    f32 = mybir.dt.float32
    i32 = mybir.dt.int32
    Alu = mybir.AluOpType
    Act = mybir.ActivationFunctionType
    Ax = mybir.AxisListType

    nc = bacc.Bacc("TRN2", target_bir_lowering=False, debug=False,
                   num_devices=NCORES)

    emb = nc.dram_tensor("emb", [N, D], f32, kind="ExternalInput").ap()
    poison = nc.dram_tensor("poison", [N, N], f32, kind="ExternalInput").ap()
    rowidx = nc.dram_tensor("rowidx", [P, T], i32, kind="ExternalInput").ap()
    boff = nc.dram_tensor("boff", [P, T], i32, kind="ExternalInput").ap()
    wvec = nc.dram_tensor("wvec", [P, T], f32, kind="ExternalInput").ap()
    out = nc.dram_tensor("partial", [1, 1], f32, kind="ExternalOutput").ap()

    with tile.TileContext(nc) as tc, ExitStack() as ctx:
        sb = ctx.enter_context(tc.tile_pool(name="sb", bufs=1))
        work = ctx.enter_context(tc.tile_pool(name="work", bufs=3))
        ps = ctx.enter_context(tc.tile_pool(name="ps", bufs=2, space="PSUM"))
        dram = ctx.enter_context(tc.tile_pool(name="dram", bufs=1, space="DRAM"))

        # ---- Phase A: distances machinery ------------------------------
        # Load E row-blocks: Esb[:, b*D:(b+1)*D] = emb[b*128:(b+1)*128, :]
        esb = sb.tile([P, NB * D], f32)
        for b in range(NB):
            nc.sync.dma_start(out=esb[:, b * D:(b + 1) * D],
                              in_=emb[b * P:(b + 1) * P, :])

        idt = sb.tile([P, P], f32)
        make_identity(nc, idt[:])

        # ET[d, n] = emb[n, d]  via PE transpose per block
        et = sb.tile([P, N], f32)
        for b in range(NB):
            pst = ps.tile([P, P], f32)
            nc.tensor.transpose(out=pst[:], in_=esb[:, b * D:(b + 1) * D],
                                identity=idt[:])
            nc.scalar.copy(out=et[:, b * P:(b + 1) * P], in_=pst[:])

        # sqrow[0, n] = sum_d emb[n, d]^2
        etsq = sb.tile([P, N], f32)
        nc.vector.tensor_tensor(out=etsq[:], in0=et[:], in1=et[:],
                                op=Alu.mult)
        ones = sb.tile([P, 1], f32)
        nc.vector.memset(ones[:], 1.0)
        ps_sq = ps.tile([1, N], f32)
        nc.tensor.matmul(out=ps_sq[:], lhsT=ones[:, 0:1], rhs=etsq[:],
                         start=True, stop=True)
        sqrow = sb.tile([1, N], f32)
        nc.scalar.copy(out=sqrow[:], in_=ps_sq[:])
        ones_r = sb.tile([1, P], f32)
        nc.vector.memset(ones_r[:], 1.0)
        etm2 = sb.tile([P, N], f32)
        nc.vector.tensor_scalar(out=etm2[:], in0=et[:], scalar1=-2.0,
                                scalar2=None, op0=Alu.mult)

        # U = sqrow_bcast - 2G  (both terms accumulated on PE);
        # V = -U - poison ; stored to DRAM
        udram = dram.tile([N * N, 1], f32)
        vdram = dram.tile([N, N], f32)
        for b in range(NB):
            ps_u = ps.tile([P, N], f32)
            nc.tensor.matmul(out=ps_u[:], lhsT=etm2[:, b * P:(b + 1) * P],
                             rhs=et[:], start=True, stop=False)
            nc.tensor.matmul(out=ps_u[:], lhsT=ones_r[0:1, :],
                             rhs=sqrow[0:1, :], start=False, stop=True)
            ub = work.tile([P, N], f32)
            nc.scalar.copy(out=ub[:], in_=ps_u[:])
            poi = work.tile([P, N], f32)
            nc.sync.dma_start(out=poi[:], in_=poison[b * P:(b + 1) * P, :])
            vb = work.tile([P, N], f32)
            nc.vector.tensor_scalar(out=vb[:], in0=ps_u[:], scalar1=-1.0,
                                    scalar2=None, op0=Alu.mult)
            nc.vector.tensor_tensor(out=vb[:], in0=vb[:], in1=poi[:],
                                    op=Alu.subtract)
            nc.sync.dma_start(
                out=udram[b * P * N:(b + 1) * P * N, :].rearrange(
                    "(p f) o -> p (f o)", p=P),
                in_=ub[:])
            nc.sync.dma_start(out=vdram[b * P:(b + 1) * P, :], in_=vb[:])

        # ---- Phase B: per-pair-tile triplet reductions -----------------
        ridx = sb.tile([P, T], i32)
        nc.sync.dma_start(out=ridx[:], in_=rowidx[:, :])
        bofs = sb.tile([P, T], i32)
        nc.sync.dma_start(out=bofs[:], in_=boff[:, :])
        wv = sb.tile([P, T], f32)
        nc.sync.dma_start(out=wv[:], in_=wvec[:, :])

        pps = sb.tile([P, T], f32)
        sh = sb.tile([P, T], f32)

        for t in range(T):
            vrow = work.tile([P, N], f32)
            nc.gpsimd.indirect_dma_start(
                out=vrow[:], out_offset=None, in_=vdram[:, :],
                in_offset=bass.IndirectOffsetOnAxis(ap=ridx[:, t:t + 1],
                                                    axis=0))
            bval = work.tile([P, 1], f32)
            nc.gpsimd.indirect_dma_start(
                out=bval[:], out_offset=None, in_=udram[:, :],
                in_offset=bass.IndirectOffsetOnAxis(ap=bofs[:, t:t + 1],
                                                    axis=0))
            bm = work.tile([P, 1], f32)
            nc.vector.tensor_scalar(out=bm[:], in0=bval[:],
                                    scalar1=float(MARGIN), scalar2=None,
                                    op0=Alu.add)
            tt = work.tile([P, N], f32)
            nc.scalar.activation(out=tt[:], in_=vrow[:], func=Act.Relu,
                                 bias=bm[:, 0:1], scale=1.0)
            le = work.tile([P, N], f32)
            nc.vector.tensor_scalar(out=le[:], in0=tt[:],
                                    scalar1=float(MARGIN), scalar2=None,
                                    op0=Alu.is_le)
            sm = work.tile([P, N], f32)
            nc.vector.tensor_tensor(out=sm[:], in0=tt[:], in1=le[:],
                                    op=Alu.mult)
            nc.vector.tensor_reduce(out=pps[:, t:t + 1], in_=sm[:],
                                    axis=Ax.X, op=Alu.max)
            g10 = work.tile([P, N], f32)
            nc.vector.tensor_scalar(out=g10[:], in0=tt[:],
                                    scalar1=float(MARGIN), scalar2=-RHO,
                                    op0=Alu.is_gt, op1=Alu.mult)
            hh = work.tile([P, N], f32)
            nc.vector.tensor_tensor(out=hh[:], in0=tt[:], in1=g10[:],
                                    op=Alu.add)
            nc.vector.tensor_reduce(out=sh[:, t:t + 1], in_=hh[:],
                                    axis=Ax.X, op=Alu.min)

        # ---- combine: ppl = w * (pps + (pps==0)*(sh+RHO)*(sh<0)) -------
        ez = sb.tile([P, T], f32)
        nc.vector.tensor_scalar(out=ez[:], in0=pps[:], scalar1=0.0,
                                scalar2=None, op0=Alu.is_equal)
        sneg = sb.tile([P, T], f32)
        nc.vector.tensor_scalar(out=sneg[:], in0=sh[:], scalar1=0.0,
                                scalar2=None, op0=Alu.is_lt)
        pph = sb.tile([P, T], f32)
        nc.vector.tensor_scalar(out=pph[:], in0=sh[:], scalar1=RHO,
                                scalar2=None, op0=Alu.add)
        nc.vector.tensor_tensor(out=pph[:], in0=pph[:], in1=sneg[:],
                                op=Alu.mult)
        nc.vector.tensor_tensor(out=pph[:], in0=pph[:], in1=ez[:],
                                op=Alu.mult)
        ppl = sb.tile([P, T], f32)
        nc.vector.tensor_tensor(out=ppl[:], in0=pps[:], in1=pph[:],
                                op=Alu.add)
        nc.vector.tensor_tensor(out=ppl[:], in0=ppl[:], in1=wv[:],
                                op=Alu.mult)
        pcol = sb.tile([P, 1], f32)
        nc.vector.tensor_reduce(out=pcol[:], in_=ppl[:], axis=Ax.X,
                                op=Alu.add)
        ps_out = ps.tile([1, 1], f32)
        nc.tensor.matmul(out=ps_out[:], lhsT=pcol[:, 0:1], rhs=ones[:, 0:1],
                         start=True, stop=True)
        osb = sb.tile([1, 1], f32)
        nc.scalar.copy(out=osb[:], in_=ps_out[:])
        nc.sync.dma_start(out=out[:, :], in_=osb[:])

    nc.compile()
    return nc


_last_results = None  # stashed BassKernelResults for profiling in test.py


def kernel(classes: np.ndarray, embeddings: np.ndarray) -> np.ndarray:
    global _last_results
    from concourse import bass_utils

    cls = np.asarray(classes).astype(np.int64)
    emb = np.ascontiguousarray(np.asarray(embeddings), dtype=np.float32)
    assert emb.shape == (N, D)

    same = cls[:, None] == cls[None, :]
    poison = (BIG * same).astype(np.float32)
    same_nd = same.copy()
    np.fill_diagonal(same_nd, False)
    ii, jj = np.nonzero(same_nd)  # valid (anchor, positive) pairs
    den = len(ii)
    if den == 0:
        return np.asarray(0.0, dtype=np.float32)

    T = max(1, math.ceil(den / (NCORES * P)))
    nslots = NCORES * T * P
    ri = np.zeros(nslots, np.int32)
    bo = np.zeros(nslots, np.int32)
    wv = np.zeros(nslots, np.float32)
    ri[:den] = ii
    bo[:den] = ii * N + jj
    wv[:den] = 1.0

    nc = _build(T)
    in_maps = []
    for c in range(NCORES):
        sl = slice(c * T * P, (c + 1) * T * P)
        in_maps.append({
            "emb": emb,
            "poison": poison,
            "rowidx": np.ascontiguousarray(ri[sl].reshape(T, P).T),
            "boff": np.ascontiguousarray(bo[sl].reshape(T, P).T),
            "wvec": np.ascontiguousarray(wv[sl].reshape(T, P).T),
        })

    res = bass_utils.run_bass_kernel_spmd(nc, in_maps,
                                          core_ids=list(range(NCORES)))
    _last_results = res
    num = float(sum(r["partial"][0, 0] for r in res.results))
    loss = num / max(den, 1)
    return np.asarray(loss, dtype=np.float32)
